# revision 9
# baseline (speedup 1.0000x reference)
"""Trainium2 Bass kernel for 6-layer GPT encoder (nn_GPT_52888227283821).

Sharding: data-parallel over batch, 2 sequences per core x 8 cores.
Layout: channel-major activations (C on partitions, tokens on free dim).
  x residual: float32r; weights/QKV/attention/MLP: bf16 matmuls, fp32 PSUM.
  Softmax without max-subtraction (|scores| small by construction);
  denominators via ones-column fused into token-major V.
LayerNorm stats computed in row domain (1 x tokens), expanded to
partitions via K=1 outer-product matmuls.
"""
import numpy as np
import ml_dtypes

import concourse.bass as bass
import concourse.mybir as mybir
from concourse import bacc, bass_utils
from concourse.tile import TileContext

F32 = mybir.dt.float32
F32R = mybir.dt.float32r
BF16 = mybir.dt.bfloat16
AF = mybir.ActivationFunctionType
OP = mybir.AluOpType

L, C, NH, HD, FF = 6, 512, 8, 64, 2048
B2 = 2
T = 1280
NT = B2 * T
P = 128
CT = C // P
FT = FF // P
NCH = NT // 512
EPS = 1e-5
QCS = [(0, 512), (512, 512), (1024, 256)]
N_CORES = 8

_CACHED_NC = None


def _ln(nc, psp, wkp, consts, x_tiles, j, out_h, lnw, lnb):
    """LayerNorm over channels for token chunk j; row-domain stats."""
    ones128, sumcol, epscol = consts["ones128"], consts["sumcol"], consts["epscol"]
    t0 = j * 512
    e1 = psp.tile([1, 512], F32, tag="prow", bufs=2, name="e1")
    e2 = psp.tile([1, 512], F32, tag="prow", bufs=2, name="e2")
    for kt in range(CT):
        xs = x_tiles[kt][:, t0:t0 + 512]
        sq = wkp.tile([128, 512], F32R, tag="sq", bufs=2, name="sq")
        nc.vector.tensor_tensor(out=sq, in0=xs.bitcast(F32), in1=xs.bitcast(F32),
                                op=OP.mult)
        nc.tensor.matmul(e1, sumcol, xs, start=(kt == 0), stop=(kt == CT - 1))
        nc.tensor.matmul(e2, sumcol, sq, start=(kt == 0), stop=(kt == CT - 1))
    # row-domain: rstd and -mean*rstd
    e1s = wkp.tile([1, 512], F32, tag="e1s", bufs=2, name="e1s")
    varr = wkp.tile([1, 512], F32, tag="varr", bufs=2, name="varr")
    rr = wkp.tile([1, 512], F32, tag="rr", bufs=2, name="rr")
    nmr = wkp.tile([1, 512], F32, tag="nmr", bufs=2, name="nmr")
    rrr = wkp.tile([1, 512], F32R, tag="rrr", bufs=2, name="rrr")
    nmrr = wkp.tile([1, 512], F32R, tag="nmrr", bufs=2, name="nmrr")
    nc.vector.tensor_copy(e1s, e1)
    nc.vector.tensor_tensor(out=varr, in0=e1s, in1=e1s, op=OP.mult)
    nc.vector.tensor_tensor(out=varr, in0=e2, in1=varr, op=OP.subtract)
    nc.scalar.activation(varr, varr, AF.Sqrt, bias=epscol[0:1, 0:1])
    nc.vector.reciprocal_approx_fast(out=rr, in_=varr)
    nc.vector.tensor_scalar(out=nmr, in0=e1s, scalar1=-1.0, scalar2=None,
                            op0=OP.mult)
    nc.vector.tensor_tensor(out=nmr, in0=nmr, in1=rr, op=OP.mult)
    nc.scalar.activation(rrr, rr, AF.Copy)
    nc.scalar.activation(nmrr, nmr, AF.Copy)
    rexp = psp.tile([128, 512], F32, tag="pexp", bufs=2, name="rexp")
    nmexp = psp.tile([128, 512], F32, tag="pexp", bufs=2, name="nmexp")
    nc.tensor.matmul(rexp, ones128[0:1, :], rrr, start=True, stop=True)
    nc.tensor.matmul(nmexp, ones128[0:1, :], nmrr, start=True, stop=True)
    for kt in range(CT):
        nc.vector.tensor_tensor(out=out_h[:, kt, :],
                                in0=x_tiles[kt][:, t0:t0 + 512].bitcast(F32),
                                in1=rexp, op=OP.mult)
        nc.vector.tensor_tensor(out=out_h[:, kt, :], in0=out_h[:, kt, :],
                                in1=nmexp, op=OP.add)
        nc.vector.tensor_scalar(out=out_h[:, kt, :], in0=out_h[:, kt, :],
                                scalar1=lnw[:, kt:kt + 1], scalar2=lnb[:, kt:kt + 1],
                                op0=OP.mult, op1=OP.add)


def build_nc():
    nc = bacc.Bacc("TRN2", target_bir_lowering=False, debug=False,
                   num_devices=N_CORES)
    img_d = nc.dram_tensor("img", (B2, C, 1024), F32R, kind="ExternalInput")
    rad_d = nc.dram_tensor("rad", (B2, C, 256), F32R, kind="ExternalInput")
    pos_d = nc.dram_tensor("posT", (C, T), F32, kind="ExternalInput")
    wq_d = nc.dram_tensor("Wq", (L, C, C), BF16, kind="ExternalInput")
    wk_d = nc.dram_tensor("Wk", (L, C, C), BF16, kind="ExternalInput")
    wv_d = nc.dram_tensor("Wv", (L, C, C), BF16, kind="ExternalInput")
    wo_d = nc.dram_tensor("Wo", (L, C, C), BF16, kind="ExternalInput")
    w1_d = nc.dram_tensor("W1", (L, C, FF), BF16, kind="ExternalInput")
    w2_d = nc.dram_tensor("W2", (L, FF, C), BF16, kind="ExternalInput")
    bq_d = nc.dram_tensor("bq", (L, C), F32, kind="ExternalInput")
    bk_d = nc.dram_tensor("bk", (L, C), F32, kind="ExternalInput")
    bv_d = nc.dram_tensor("bv", (L, C), BF16, kind="ExternalInput")
    bo_d = nc.dram_tensor("bo", (L, C), BF16, kind="ExternalInput")
    b1_d = nc.dram_tensor("b1", (L, FF), F32, kind="ExternalInput")
    b2_d = nc.dram_tensor("b2", (L, C), BF16, kind="ExternalInput")
    l1w_d = nc.dram_tensor("ln1_w", (L, C), F32, kind="ExternalInput")
    l1b_d = nc.dram_tensor("ln1_b", (L, C), F32, kind="ExternalInput")
    l2w_d = nc.dram_tensor("ln2_w", (L, C), F32, kind="ExternalInput")
    l2b_d = nc.dram_tensor("ln2_b", (L, C), F32, kind="ExternalInput")
    lfw_d = nc.dram_tensor("lnf_w", (C,), F32, kind="ExternalInput")
    lfb_d = nc.dram_tensor("lnf_b", (C,), F32, kind="ExternalInput")
    out_d = nc.dram_tensor("out_xT", (B2, C, T), F32, kind="ExternalOutput")

    with TileContext(nc) as tc:
        with tc.tile_pool(name="persist", bufs=1) as pp:
            x_tiles = [pp.tile([128, NT], F32R, tag=f"x{kt}", name=f"x{kt}")
                       for kt in range(CT)]
            q_tiles = [pp.tile([128, NT], BF16, tag=f"q{kt}", name=f"q{kt}")
                       for kt in range(CT)]
            k_tiles = [pp.tile([128, NT], BF16, tag=f"k{kt}", name=f"k{kt}")
                       for kt in range(CT)]
            vones = pp.tile([128, 20, NH, HD + 1], BF16, tag="vones")
            ones128 = pp.tile([1, 128], F32R, tag="ones128")
            sumcol = pp.tile([128, 1], F32R, tag="sumcol")
            epscol = pp.tile([128, 1], F32, tag="epscol")
            ones_bf = pp.tile([1, 512], BF16, tag="ones_bf")
            lfw_t = pp.tile([128, CT], F32, tag="lfw")
            lfb_t = pp.tile([128, CT], F32, tag="lfb")
            scr128 = pp.tile([1, 128], F32, tag="scr128")
            scrcol = pp.tile([128, 1], F32, tag="scrcol")
            nc.vector.memset(scr128, 1.0)
            nc.scalar.activation(ones128, scr128, AF.Copy)
            nc.vector.memset(scrcol, 1.0 / C)
            nc.scalar.activation(sumcol, scrcol, AF.Copy)
            nc.vector.memset(epscol, EPS)
            nc.vector.memset(ones_bf, 1.0)
            nc.vector.memset(vones, 1.0)
            nc.sync.dma_start(out=lfw_t, in_=lfw_d.rearrange("(kt p) -> p kt", p=128))
            nc.sync.dma_start(out=lfb_t, in_=lfb_d.rearrange("(kt p) -> p kt", p=128))
            consts = dict(ones128=ones128, sumcol=sumcol, epscol=epscol)

            with tc.tile_pool(name="init", bufs=1) as ip:
                for kt in range(CT):
                    pos_t = ip.tile([128, T], F32, tag=f"pos{kt}", name=f"pos{kt}")
                    nc.sync.dma_start(out=pos_t, in_=pos_d[kt * 128:(kt + 1) * 128, :])
                    for s in range(B2):
                        g = s * T
                        nc.sync.dma_start(out=x_tiles[kt][:, g:g + 1024],
                                          in_=img_d[s, kt * 128:(kt + 1) * 128, :])
                        nc.sync.dma_start(out=x_tiles[kt][:, g + 1024:g + 1280],
                                          in_=rad_d[s, kt * 128:(kt + 1) * 128, :])
                        nc.vector.tensor_tensor(
                            out=x_tiles[kt][:, g:g + T],
                            in0=x_tiles[kt][:, g:g + T].bitcast(F32),
                            in1=pos_t, op=OP.add)

            with tc.tile_pool(name="wts", bufs=1) as wp:
                for l in range(L):
                    wq_t = wp.tile([128, CT, 512], BF16, tag="wq", name="wq_t")
                    wk_t = wp.tile([128, CT, 512], BF16, tag="wk", name="wk_t")
                    wv_t = wp.tile([128, CT, 512], BF16, tag="wv", name="wv_t")
                    wo_t = wp.tile([128, CT, 512], BF16, tag="wo", name="wo_t")
                    w2_t = wp.tile([128, FT, 512], BF16, tag="w2", name="w2_t")
                    for wt, wd in ((wq_t, wq_d), (wk_t, wk_d), (wv_t, wv_d),
                                   (wo_t, wo_d)):
                        nc.sync.dma_start(
                            out=wt, in_=wd[l].rearrange("(kt p) n -> p kt n", p=128))
                    nc.sync.dma_start(
                        out=w2_t, in_=w2_d[l].rearrange("(kt p) n -> p kt n", p=128))
                    bq_t = wp.tile([128, CT], F32, tag="bq", name="bq_t")
                    bk_t = wp.tile([128, CT], F32, tag="bk", name="bk_t")
                    b1_t = wp.tile([128, FT], F32, tag="b1", name="b1_t")
                    nc.sync.dma_start(out=bq_t, in_=bq_d[l].rearrange("(kt p) -> p kt", p=128))
                    nc.sync.dma_start(out=bk_t, in_=bk_d[l].rearrange("(kt p) -> p kt", p=128))
                    nc.sync.dma_start(out=b1_t, in_=b1_d[l].rearrange("(kt p) -> p kt", p=128))
                    bv_r = wp.tile([1, 512], BF16, tag="bvr", name="bv_r")
                    bo_r = wp.tile([1, 512], BF16, tag="bor", name="bo_r")
                    b2_r = wp.tile([1, 512], BF16, tag="b2r", name="b2_r")
                    nc.sync.dma_start(out=bv_r, in_=bv_d[l:l + 1, :])
                    nc.sync.dma_start(out=bo_r, in_=bo_d[l:l + 1, :])
                    nc.sync.dma_start(out=b2_r, in_=b2_d[l:l + 1, :])
                    lnt = {}
                    for nm, dd in (("l1w", l1w_d), ("l1b", l1b_d),
                                   ("l2w", l2w_d), ("l2b", l2b_d)):
                        lnt[nm] = wp.tile([128, CT], F32, tag=nm, name=nm)
                        nc.sync.dma_start(out=lnt[nm],
                                          in_=dd[l].rearrange("(kt p) -> p kt", p=128))

                    # ---------- Phase A: LN1 + QKV ----------
                    with tc.tile_pool(name=f"qkv{l}", bufs=1) as wkp, \
                         tc.tile_pool(name=f"qkvp{l}", bufs=1, space="PSUM") as psp:
                        for j in range(NCH):
                            t0 = j * 512
                            h = wkp.tile([128, CT, 512], BF16, tag="h", bufs=2,
                                         name="h")
                            _ln(nc, psp, wkp, consts, x_tiles, j, h,
                                lnt["l1w"], lnt["l1b"])
                            for (wt, bt, dst) in ((wq_t, bq_t, q_tiles),
                                                  (wk_t, bk_t, k_tiles)):
                                for m in range(CT):
                                    pq = psp.tile([128, 512], F32, tag="pa",
                                                  bufs=4, name="pq")
                                    for kt in range(CT):
                                        nc.tensor.matmul(
                                            pq, wt[:, kt, m * 128:(m + 1) * 128],
                                            h[:, kt, :],
                                            start=(kt == 0), stop=(kt == CT - 1))
                                    nc.vector.tensor_scalar(
                                        out=dst[m][:, t0:t0 + 512], in0=pq,
                                        scalar1=bt[:, m:m + 1], scalar2=None,
                                        op0=OP.add)
                            for cc in range(4):
                                gc = j * 4 + cc
                                pv = psp.tile([128, 512], F32, tag="pa", bufs=4,
                                              name="pv")
                                for kt in range(CT):
                                    nc.tensor.matmul(
                                        pv, h[:, kt, cc * 128:(cc + 1) * 128],
                                        wv_t[:, kt, :], start=(kt == 0), stop=False)
                                nc.tensor.matmul(pv, ones_bf[0:1, 0:128], bv_r,
                                                 start=False, stop=True)
                                nc.vector.tensor_copy(
                                    out=vones[:, gc, :, 0:HD],
                                    in_=pv.rearrange("p (nh hd) -> p nh hd", nh=NH))

                    # ---------- Phase B: attention + O-proj ----------
                    with tc.tile_pool(name=f"att{l}", bufs=1) as wkp, \
                         tc.tile_pool(name=f"attp{l}", bufs=1, space="PSUM") as psp:
                        for s in range(B2):
                            for (q0, qs) in QCS:
                                yq = wkp.tile([128, CT, 512], BF16, tag="yq",
                                              bufs=2, name="yq")
                                for hh in range(NH):
                                    po = (hh % 2) * 64
                                    aT = wkp.tile([128, 10, 512], BF16, tag="aT",
                                                  bufs=2, name="aT")
                                    for kcp in range(5):
                                        ps2 = psp.tile([128, 1024], F32, tag="sc2",
                                                       bufs=2, name="ps2")
                                        for half in range(2):
                                            kc = 2 * kcp + half
                                            nc.tensor.matmul(
                                                ps2[:, half * qs:half * qs + qs],
                                                k_tiles[hh // 2][po:po + 64,
                                                    s * T + kc * 128:s * T + (kc + 1) * 128],
                                                q_tiles[hh // 2][po:po + 64,
                                                    s * T + q0:s * T + q0 + qs],
                                                start=True, stop=True)
                                        nc.scalar.activation(
                                            aT[:, 2 * kcp:2 * kcp + 2, 0:qs],
                                            ps2[:, 0:2 * qs],
                                            AF.Exp, scale=1.0 / np.sqrt(HD))
                                    pav = psp.tile([65, 512], F32, tag="p65",
                                                   bufs=2, name="pav")
                                    for kc in range(10):
                                        nc.tensor.matmul(
                                            pav[:, 0:qs], vones[:, s * 10 + kc, hh, :],
                                            aT[:, kc, 0:qs],
                                            start=(kc == 0), stop=(kc == 9))
                                    rec = wkp.tile([1, 512], F32, tag="rec",
                                                   bufs=2, name="rec")
                                    recr = wkp.tile([1, 512], F32R, tag="recr",
                                                    bufs=2, name="recr")
                                    den = wkp.tile([1, 512], F32, tag="den",
                                                   bufs=2, name="den")
                                    nc.vector.tensor_copy(out=den[:, 0:qs],
                                                          in_=pav[64:65, 0:qs])
                                    nc.vector.reciprocal_approx_fast(
                                        out=rec[:, 0:qs], in_=den[:, 0:qs])
                                    nc.scalar.activation(recr[:, 0:qs], rec[:, 0:qs],
                                                         AF.Copy)
                                    dexp = psp.tile([64, 512], F32, tag="pa512",
                                                    bufs=2, name="dexp")
                                    nc.tensor.matmul(dexp[:, 0:qs], ones128[0:1, 0:64],
                                                     recr[0:1, 0:qs], start=True,
                                                     stop=True)
                                    dexps = wkp.tile([64, 512], F32, tag="dexps",
                                                     bufs=2, name="dexps")
                                    nc.vector.tensor_copy(out=dexps[:, 0:qs],
                                                          in_=dexp[:, 0:qs])
                                    nc.vector.tensor_tensor(
                                        out=yq[po:po + 64, hh // 2, 0:qs],
                                        in0=pav[0:64, 0:qs], in1=dexps[:, 0:qs],
                                        op=OP.mult)
                                for m in range(CT):
                                    pp2 = psp.tile([128, 512], F32, tag="pa512",
                                                   bufs=2, name="pp2")
                                    for kt in range(CT):
                                        nc.tensor.matmul(
                                            pp2[:, 0:qs], wo_t[:, kt, m * 128:(m + 1) * 128],
                                            yq[:, kt, 0:qs], start=(kt == 0), stop=False)
                                    nc.tensor.matmul(pp2[:, 0:qs],
                                                     bo_r[0:1, m * 128:(m + 1) * 128],
                                                     ones_bf[0:1, 0:qs],
                                                     start=False, stop=True)
                                    g = s * T + q0
                                    nc.vector.tensor_tensor(
                                        out=x_tiles[m][:, g:g + qs],
                                        in0=pp2[:, 0:qs],
                                        in1=x_tiles[m][:, g:g + qs].bitcast(F32),
                                        op=OP.add)

                    # ---------- Phase C: LN2 + MLP ----------
                    with tc.tile_pool(name=f"mlp{l}", bufs=1) as wkp, \
                         tc.tile_pool(name=f"mlpp{l}", bufs=1, space="PSUM") as psp:
                        for j in range(NCH):
                            t0 = j * 512
                            h2 = wkp.tile([128, CT, 512], BF16, tag="h", bufs=2,
                                          name="h2")
                            _ln(nc, psp, wkp, consts, x_tiles, j, h2,
                                lnt["l2w"], lnt["l2b"])
                            h1 = [wkp.tile([128, 512], BF16, tag=f"h1_{m}",
                                           name=f"h1_{m}") for m in range(FT)]
                            for qq in range(4):
                                w1q = wkp.tile([128, CT, 512], BF16, tag="w1q",
                                               bufs=2, name="w1q")
                                nc.sync.dma_start(
                                    out=w1q,
                                    in_=w1_d[l].rearrange("(kt p) n -> p kt n",
                                                          p=128)[:, :, qq * 512:(qq + 1) * 512])
                                for mm in range(4):
                                    m = qq * 4 + mm
                                    ph = psp.tile([128, 512], F32, tag="pa",
                                                  bufs=4, name="ph")
                                    for kt in range(CT):
                                        nc.tensor.matmul(
                                            ph, w1q[:, kt, mm * 128:(mm + 1) * 128],
                                            h2[:, kt, :],
                                            start=(kt == 0), stop=(kt == CT - 1))
                                    nc.vector.tensor_scalar(
                                        out=h1[m], in0=ph,
                                        scalar1=b1_t[:, m:m + 1], scalar2=0.0,
                                        op0=OP.add, op1=OP.max)
                            for m2 in range(CT):
                                pw = psp.tile([128, 512], F32, tag="pa", bufs=4,
                                              name="pw")
                                for f in range(FT):
                                    nc.tensor.matmul(
                                        pw, w2_t[:, f, m2 * 128:(m2 + 1) * 128],
                                        h1[f], start=(f == 0), stop=False)
                                nc.tensor.matmul(pw, b2_r[0:1, m2 * 128:(m2 + 1) * 128],
                                                 ones_bf[0:1, :], start=False, stop=True)
                                nc.vector.tensor_tensor(
                                    out=x_tiles[m2][:, t0:t0 + 512],
                                    in0=pw,
                                    in1=x_tiles[m2][:, t0:t0 + 512].bitcast(F32),
                                    op=OP.add)

            with tc.tile_pool(name="fin", bufs=1) as wkp, \
                 tc.tile_pool(name="finp", bufs=1, space="PSUM") as psp:
                for j in range(NCH):
                    fin = wkp.tile([128, CT, 512], F32, tag="fin", bufs=2,
                                   name="fin")
                    _ln(nc, psp, wkp, consts, x_tiles, j, fin, lfw_t, lfb_t)
                    t0 = j * 512
                    for kt in range(CT):
                        lo, hi = t0, t0 + 512
                        if hi <= T:
                            nc.sync.dma_start(
                                out=out_d[0, kt * 128:(kt + 1) * 128, lo:hi],
                                in_=fin[:, kt, :])
                        elif lo >= T:
                            nc.sync.dma_start(
                                out=out_d[1, kt * 128:(kt + 1) * 128, lo - T:hi - T],
                                in_=fin[:, kt, :])
                        else:
                            cut = T - lo
                            nc.sync.dma_start(
                                out=out_d[0, kt * 128:(kt + 1) * 128, lo:T],
                                in_=fin[:, kt, 0:cut])
                            nc.sync.dma_start(
                                out=out_d[1, kt * 128:(kt + 1) * 128, 0:512 - cut],
                                in_=fin[:, kt, cut:512])
    nc.compile()
    return nc


def _get_nc():
    global _CACHED_NC
    if _CACHED_NC is None:
        _CACHED_NC = build_nc()
    return _CACHED_NC


def run(inputs, trace=False):
    nc = _get_nc()
    bf = lambda a: np.asarray(a).astype(ml_dtypes.bfloat16)
    f32 = lambda a: np.ascontiguousarray(np.asarray(a), dtype=np.float32)
    img = f32(inputs["image_tensor"]).reshape(16, C, 1024)
    rad = f32(inputs["radar_tensor"]).reshape(16, C, 256)
    posT = np.ascontiguousarray(f32(inputs["pos_emb"])[0].T)
    shared = dict(
        posT=posT,
        Wq=bf(inputs["Wq"]), Wk=bf(inputs["Wk"]), Wv=bf(inputs["Wv"]),
        Wo=bf(inputs["Wo"]), W1=bf(inputs["W1"]), W2=bf(inputs["W2"]),
        bq=f32(inputs["bq"]), bk=f32(inputs["bk"]), bv=bf(inputs["bv"]),
        bo=bf(inputs["bo"]), b1=f32(inputs["b1"]), b2=bf(inputs["b2"]),
        ln1_w=f32(inputs["ln1_w"]), ln1_b=f32(inputs["ln1_b"]),
        ln2_w=f32(inputs["ln2_w"]), ln2_b=f32(inputs["ln2_b"]),
        lnf_w=f32(inputs["lnf_w"]), lnf_b=f32(inputs["lnf_b"]),
    )
    in_maps = []
    for c in range(N_CORES):
        m = dict(shared)
        m["img"] = np.ascontiguousarray(img[2 * c:2 * c + 2])
        m["rad"] = np.ascontiguousarray(rad[2 * c:2 * c + 2])
        in_maps.append(m)
    res = bass_utils.run_bass_kernel_spmd(nc, in_maps,
                                          core_ids=list(range(N_CORES)),
                                          trace=trace)
    xT = np.concatenate([res.results[c]["out_xT"] for c in range(N_CORES)], 0)
    x = np.ascontiguousarray(xT.transpose(0, 2, 1))
    img_out = x[:, :1024, :].reshape(16, C, 16, 64)
    rad_out = x[:, 1024:, :].reshape(16, C, 16, 16)
    return (img_out, rad_out), res


def kernel(**inputs):
    out, _ = run(inputs, trace=False)
    return out


# revision 12
# speedup vs baseline: 1.0085x; 1.0085x over previous
"""Trainium2 Bass kernel for 6-layer GPT encoder (nn_GPT_52888227283821).

Sharding: data-parallel over batch, 2 sequences per core x 8 cores.
Layout: channel-major activations (C on partitions, tokens on free dim).
  x residual: float32r; weights/QKV/attention/MLP: bf16 matmuls, fp32 PSUM.
  Softmax without max-subtraction (|scores| small by construction);
  denominators via ones-column fused into token-major V.
LayerNorm stats computed in row domain (1 x tokens), expanded to
partitions via K=1 outer-product matmuls.
"""
import numpy as np
import ml_dtypes

import concourse.bass as bass
import concourse.mybir as mybir
from concourse import bacc, bass_utils
from concourse.tile import TileContext

F32 = mybir.dt.float32
F32R = mybir.dt.float32r
BF16 = mybir.dt.bfloat16
AF = mybir.ActivationFunctionType
OP = mybir.AluOpType

L, C, NH, HD, FF = 6, 512, 8, 64, 2048
B2 = 2
T = 1280
NT = B2 * T
P = 128
CT = C // P
FT = FF // P
NCH = NT // 512
EPS = 1e-5
QCS = [(0, 512), (512, 512), (1024, 256)]
N_CORES = 8

_CACHED_NC = {}


def _ln(nc, psp, wkp, consts, x_tiles, j, out_h, lnw, lnb, apply_lnwb=True):
    """LayerNorm over channels for token chunk j; row-domain stats."""
    ones128, sumcol, epscol = consts["ones128"], consts["sumcol"], consts["epscol"]
    t0 = j * 512
    e1 = psp.tile([1, 512], F32, tag="prow", bufs=2, name="e1")
    e2 = psp.tile([1, 512], F32, tag="prow", bufs=2, name="e2")
    for kt in range(CT):
        xs = x_tiles[kt][:, t0:t0 + 512]
        sq = wkp.tile([128, 512], F32R, tag="sq", bufs=2, name="sq")
        nc.vector.tensor_tensor(out=sq, in0=xs.bitcast(F32), in1=xs.bitcast(F32),
                                op=OP.mult)
        nc.tensor.matmul(e1, sumcol, xs, start=(kt == 0), stop=(kt == CT - 1))
        nc.tensor.matmul(e2, sumcol, sq, start=(kt == 0), stop=(kt == CT - 1))
    # row-domain: rstd and -mean*rstd
    e1s = wkp.tile([1, 512], F32, tag="e1s", bufs=2, name="e1s")
    varr = wkp.tile([1, 512], F32, tag="varr", bufs=2, name="varr")
    rr = wkp.tile([1, 512], F32, tag="rr", bufs=2, name="rr")
    nmr = wkp.tile([1, 512], F32, tag="nmr", bufs=2, name="nmr")
    rrr = wkp.tile([1, 512], F32R, tag="rrr", bufs=2, name="rrr")
    nmrr = wkp.tile([1, 512], F32R, tag="nmrr", bufs=2, name="nmrr")
    nc.vector.tensor_copy(e1s, e1)
    nc.vector.tensor_tensor(out=varr, in0=e1s, in1=e1s, op=OP.mult)
    nc.vector.tensor_tensor(out=varr, in0=e2, in1=varr, op=OP.subtract)
    nc.scalar.activation(varr, varr, AF.Sqrt, bias=epscol[0:1, 0:1])
    nc.vector.reciprocal_approx_fast(out=rr, in_=varr)
    nc.vector.tensor_scalar(out=nmr, in0=e1s, scalar1=-1.0, scalar2=None,
                            op0=OP.mult)
    nc.vector.tensor_tensor(out=nmr, in0=nmr, in1=rr, op=OP.mult)
    nc.scalar.activation(rrr, rr, AF.Copy)
    nc.scalar.activation(nmrr, nmr, AF.Copy)
    rexp = psp.tile([128, 512], F32, tag="pexp", bufs=2, name="rexp")
    nmexp = psp.tile([128, 512], F32, tag="pexp", bufs=2, name="nmexp")
    nc.tensor.matmul(rexp, ones128[0:1, :], rrr, start=True, stop=True)
    nc.tensor.matmul(nmexp, ones128[0:1, :], nmrr, start=True, stop=True)
    for kt in range(CT):
        nc.vector.tensor_tensor(out=out_h[:, kt, :],
                                in0=x_tiles[kt][:, t0:t0 + 512].bitcast(F32),
                                in1=rexp, op=OP.mult)
        nc.vector.tensor_tensor(out=out_h[:, kt, :], in0=out_h[:, kt, :],
                                in1=nmexp, op=OP.add)
        if apply_lnwb:
            nc.vector.tensor_scalar(out=out_h[:, kt, :], in0=out_h[:, kt, :],
                                    scalar1=lnw[:, kt:kt + 1], scalar2=lnb[:, kt:kt + 1],
                                    op0=OP.mult, op1=OP.add)


def build_nc(fold_bias=True, apply_lnwb=True):
    nc = bacc.Bacc("TRN2", target_bir_lowering=False, debug=False,
                   num_devices=N_CORES)
    img_d = nc.dram_tensor("img", (B2, C, 1024), F32R, kind="ExternalInput")
    rad_d = nc.dram_tensor("rad", (B2, C, 256), F32R, kind="ExternalInput")
    pos_d = nc.dram_tensor("posT", (C, T), F32, kind="ExternalInput")
    wq_d = nc.dram_tensor("Wq", (L, C, C), BF16, kind="ExternalInput")
    wk_d = nc.dram_tensor("Wk", (L, C, C), BF16, kind="ExternalInput")
    wv_d = nc.dram_tensor("Wv", (L, C, C), BF16, kind="ExternalInput")
    wo_d = nc.dram_tensor("Wo", (L, C, C), BF16, kind="ExternalInput")
    w1_d = nc.dram_tensor("W1", (L, C, FF), BF16, kind="ExternalInput")
    w2_d = nc.dram_tensor("W2", (L, FF, C), BF16, kind="ExternalInput")
    bq_d = nc.dram_tensor("bq", (L, C), F32, kind="ExternalInput")
    bk_d = nc.dram_tensor("bk", (L, C), F32, kind="ExternalInput")
    bv_d = nc.dram_tensor("bv", (L, C), BF16, kind="ExternalInput")
    bo_d = nc.dram_tensor("bo", (L, C), BF16, kind="ExternalInput")
    b1_d = nc.dram_tensor("b1", (L, FF), F32, kind="ExternalInput")
    b2_d = nc.dram_tensor("b2", (L, C), BF16, kind="ExternalInput")
    l1w_d = nc.dram_tensor("ln1_w", (L, C), F32, kind="ExternalInput")
    l1b_d = nc.dram_tensor("ln1_b", (L, C), F32, kind="ExternalInput")
    l2w_d = nc.dram_tensor("ln2_w", (L, C), F32, kind="ExternalInput")
    l2b_d = nc.dram_tensor("ln2_b", (L, C), F32, kind="ExternalInput")
    lfw_d = nc.dram_tensor("lnf_w", (C,), F32, kind="ExternalInput")
    lfb_d = nc.dram_tensor("lnf_b", (C,), F32, kind="ExternalInput")
    out_d = nc.dram_tensor("out_xT", (B2, C, T), F32, kind="ExternalOutput")

    with TileContext(nc) as tc:
        with tc.tile_pool(name="persist", bufs=1) as pp:
            x_tiles = [pp.tile([128, NT], F32R, tag=f"x{kt}", name=f"x{kt}")
                       for kt in range(CT)]
            q_tiles = [pp.tile([128, NT], BF16, tag=f"q{kt}", name=f"q{kt}")
                       for kt in range(CT)]
            k_tiles = [pp.tile([128, NT], BF16, tag=f"k{kt}", name=f"k{kt}")
                       for kt in range(CT)]
            vones = pp.tile([128, 20, NH, HD + 1], BF16, tag="vones")
            ones128 = pp.tile([1, 128], F32R, tag="ones128")
            sumcol = pp.tile([128, 1], F32R, tag="sumcol")
            epscol = pp.tile([128, 1], F32, tag="epscol")
            ones_bf = pp.tile([1, 512], BF16, tag="ones_bf")
            lfw_t = pp.tile([128, CT], F32, tag="lfw")
            lfb_t = pp.tile([128, CT], F32, tag="lfb")
            scr128 = pp.tile([1, 128], F32, tag="scr128")
            scrcol = pp.tile([128, 1], F32, tag="scrcol")
            nc.vector.memset(scr128, 1.0)
            nc.scalar.activation(ones128, scr128, AF.Copy)
            nc.vector.memset(scrcol, 1.0 / C)
            nc.scalar.activation(sumcol, scrcol, AF.Copy)
            nc.vector.memset(epscol, EPS)
            nc.vector.memset(ones_bf, 1.0)
            nc.vector.memset(vones, 1.0)
            nc.sync.dma_start(out=lfw_t, in_=lfw_d.rearrange("(kt p) -> p kt", p=128))
            nc.sync.dma_start(out=lfb_t, in_=lfb_d.rearrange("(kt p) -> p kt", p=128))
            consts = dict(ones128=ones128, sumcol=sumcol, epscol=epscol)

            with tc.tile_pool(name="init", bufs=1) as ip:
                for kt in range(CT):
                    pos_t = ip.tile([128, T], F32, tag=f"pos{kt}", name=f"pos{kt}")
                    nc.sync.dma_start(out=pos_t, in_=pos_d[kt * 128:(kt + 1) * 128, :])
                    for s in range(B2):
                        g = s * T
                        nc.sync.dma_start(out=x_tiles[kt][:, g:g + 1024],
                                          in_=img_d[s, kt * 128:(kt + 1) * 128, :])
                        nc.sync.dma_start(out=x_tiles[kt][:, g + 1024:g + 1280],
                                          in_=rad_d[s, kt * 128:(kt + 1) * 128, :])
                        nc.vector.tensor_tensor(
                            out=x_tiles[kt][:, g:g + T],
                            in0=x_tiles[kt][:, g:g + T].bitcast(F32),
                            in1=pos_t, op=OP.add)

            with tc.tile_pool(name="wts", bufs=1) as wp:
                for l in range(L):
                    wq_t = wp.tile([128, CT, 512], BF16, tag="wq", name="wq_t")
                    wk_t = wp.tile([128, CT, 512], BF16, tag="wk", name="wk_t")
                    wv_t = wp.tile([128, CT, 512], BF16, tag="wv", name="wv_t")
                    wo_t = wp.tile([128, CT, 512], BF16, tag="wo", name="wo_t")
                    w2_t = wp.tile([128, FT, 512], BF16, tag="w2", name="w2_t")
                    for wt, wd in ((wq_t, wq_d), (wk_t, wk_d), (wv_t, wv_d),
                                   (wo_t, wo_d)):
                        nc.sync.dma_start(
                            out=wt, in_=wd[l].rearrange("(kt p) n -> p kt n", p=128))
                    nc.sync.dma_start(
                        out=w2_t, in_=w2_d[l].rearrange("(kt p) n -> p kt n", p=128))
                    bq_t = wp.tile([128, CT], F32, tag="bq", name="bq_t")
                    bk_t = wp.tile([128, CT], F32, tag="bk", name="bk_t")
                    b1_t = wp.tile([128, FT], F32, tag="b1", name="b1_t")
                    nc.sync.dma_start(out=bq_t, in_=bq_d[l].rearrange("(kt p) -> p kt", p=128))
                    nc.sync.dma_start(out=bk_t, in_=bk_d[l].rearrange("(kt p) -> p kt", p=128))
                    nc.sync.dma_start(out=b1_t, in_=b1_d[l].rearrange("(kt p) -> p kt", p=128))
                    bv_r = wp.tile([1, 512], BF16, tag="bvr", name="bv_r")
                    bo_r = wp.tile([1, 512], BF16, tag="bor", name="bo_r")
                    b2_r = wp.tile([1, 512], BF16, tag="b2r", name="b2_r")
                    nc.sync.dma_start(out=bv_r, in_=bv_d[l:l + 1, :])
                    nc.sync.dma_start(out=bo_r, in_=bo_d[l:l + 1, :])
                    nc.sync.dma_start(out=b2_r, in_=b2_d[l:l + 1, :])
                    lnt = {}
                    for nm, dd in (("l1w", l1w_d), ("l1b", l1b_d),
                                   ("l2w", l2w_d), ("l2b", l2b_d)):
                        lnt[nm] = wp.tile([128, CT], F32, tag=nm, name=nm)
                        nc.sync.dma_start(out=lnt[nm],
                                          in_=dd[l].rearrange("(kt p) -> p kt", p=128))

                    # ---------- Phase A: LN1 + QKV ----------
                    with tc.tile_pool(name=f"qkv{l}", bufs=1) as wkp, \
                         tc.tile_pool(name=f"qkvp{l}", bufs=1, space="PSUM") as psp:
                        for j in range(NCH):
                            t0 = j * 512
                            h = wkp.tile([128, CT, 512], BF16, tag="h", bufs=2,
                                         name="h")
                            _ln(nc, psp, wkp, consts, x_tiles, j, h,
                                lnt["l1w"], lnt["l1b"], apply_lnwb)
                            for (wt, bt, dst) in ((wq_t, bq_t, q_tiles),
                                                  (wk_t, bk_t, k_tiles)):
                                for m in range(CT):
                                    pq = psp.tile([128, 512], F32, tag="pa",
                                                  bufs=4, name="pq")
                                    for kt in range(CT):
                                        nc.tensor.matmul(
                                            pq, wt[:, kt, m * 128:(m + 1) * 128],
                                            h[:, kt, :],
                                            start=(kt == 0), stop=(kt == CT - 1))
                                    nc.scalar.activation(
                                        dst[m][:, t0:t0 + 512], pq, AF.Identity,
                                        bias=bt[:, m:m + 1])
                            for cc in range(4):
                                gc = j * 4 + cc
                                pv = psp.tile([128, 512], F32, tag="pa", bufs=4,
                                              name="pv")
                                for kt in range(CT):
                                    nc.tensor.matmul(
                                        pv, h[:, kt, cc * 128:(cc + 1) * 128],
                                        wv_t[:, kt, :], start=(kt == 0),
                                        stop=(not fold_bias and kt == CT - 1))
                                if fold_bias:
                                    nc.tensor.matmul(pv, ones_bf[0:1, 0:128], bv_r,
                                                     start=False, stop=True)
                                nc.scalar.activation(
                                    vones[:, gc, :, 0:HD],
                                    pv.rearrange("p (nh hd) -> p nh hd", nh=NH),
                                    AF.Copy)

                    # ---------- Phase B: attention + O-proj ----------
                    with tc.tile_pool(name=f"att{l}", bufs=1) as wkp, \
                         tc.tile_pool(name=f"attp{l}", bufs=1, space="PSUM") as psp:
                        for s in range(B2):
                            for (q0, qs) in QCS:
                                yq = wkp.tile([128, CT, 512], BF16, tag="yq",
                                              bufs=2, name="yq")
                                for hp in range(NH // 2):
                                    den_row = wkp.tile([1, 2 * 512], F32, tag="denrow",
                                                       bufs=2, name="den_row")
                                    rec_row = wkp.tile([1, 2 * 512], F32, tag="recrow",
                                                       bufs=2, name="rec_row")
                                    recr_row = wkp.tile([1, 2 * 512], F32R, tag="recrrow",
                                                        bufs=2, name="recr_row")
                                    pavs = []
                                    for hi in range(2):
                                        hh = 2 * hp + hi
                                        po = (hh % 2) * 64
                                        aT = wkp.tile([128, 10, 512], BF16, tag="aT",
                                                      bufs=2, name="aT")
                                        for kcp in range(5):
                                            ps2 = psp.tile([128, 1024], F32, tag="sc2",
                                                           bufs=2, name="ps2")
                                            for half in range(2):
                                                kc = 2 * kcp + half
                                                nc.tensor.matmul(
                                                    ps2[:, half * qs:half * qs + qs],
                                                    k_tiles[hh // 2][po:po + 64,
                                                        s * T + kc * 128:s * T + (kc + 1) * 128],
                                                    q_tiles[hh // 2][po:po + 64,
                                                        s * T + q0:s * T + q0 + qs],
                                                    start=True, stop=True)
                                            nc.scalar.activation(
                                                aT[:, 2 * kcp:2 * kcp + 2, 0:qs],
                                                ps2[:, 0:2 * qs],
                                                AF.Exp, scale=1.0 / np.sqrt(HD))
                                        pav = psp.tile([65, 512], F32, tag="p65",
                                                       bufs=2, name="pav")
                                        pavs.append(pav)
                                        for kc in range(10):
                                            nc.tensor.matmul(
                                                pav[:, 0:qs], vones[:, s * 10 + kc, hh, :],
                                                aT[:, kc, 0:qs],
                                                start=(kc == 0), stop=(kc == 9))
                                        nc.vector.tensor_copy(
                                            out=den_row[0:1, hi * qs:hi * qs + qs],
                                            in_=pav[64:65, 0:qs])
                                    nc.vector.reciprocal_approx_fast(
                                        out=rec_row[:, 0:2 * qs], in_=den_row[:, 0:2 * qs])
                                    nc.scalar.activation(recr_row[:, 0:2 * qs],
                                                         rec_row[:, 0:2 * qs], AF.Copy)
                                    for hi in range(2):
                                        hh = 2 * hp + hi
                                        po = (hh % 2) * 64
                                        pav = pavs[hi]
                                        dexp = psp.tile([64, 512], F32, tag="pa512",
                                                        bufs=2, name="dexp")
                                        nc.tensor.matmul(dexp[:, 0:qs], ones128[0:1, 0:64],
                                                         recr_row[0:1, hi * qs:hi * qs + qs],
                                                         start=True, stop=True)
                                        dexps = wkp.tile([64, 512], F32, tag="dexps",
                                                         bufs=2, name="dexps")
                                        nc.vector.tensor_copy(out=dexps[:, 0:qs],
                                                              in_=dexp[:, 0:qs])
                                        nc.vector.tensor_tensor(
                                            out=yq[po:po + 64, hh // 2, 0:qs],
                                            in0=pav[0:64, 0:qs], in1=dexps[:, 0:qs],
                                            op=OP.mult)
                                for m in range(CT):
                                    pp2 = psp.tile([128, 512], F32, tag="pa512",
                                                   bufs=2, name="pp2")
                                    for kt in range(CT):
                                        nc.tensor.matmul(
                                            pp2[:, 0:qs], wo_t[:, kt, m * 128:(m + 1) * 128],
                                            yq[:, kt, 0:qs], start=(kt == 0),
                                            stop=(not fold_bias and kt == CT - 1))
                                    if fold_bias:
                                        nc.tensor.matmul(pp2[:, 0:qs],
                                                         bo_r[0:1, m * 128:(m + 1) * 128],
                                                         ones_bf[0:1, 0:qs],
                                                         start=False, stop=True)
                                    g = s * T + q0
                                    nc.vector.tensor_tensor(
                                        out=x_tiles[m][:, g:g + qs],
                                        in0=pp2[:, 0:qs],
                                        in1=x_tiles[m][:, g:g + qs].bitcast(F32),
                                        op=OP.add)

                    # ---------- Phase C: LN2 + MLP ----------
                    with tc.tile_pool(name=f"mlp{l}", bufs=1) as wkp, \
                         tc.tile_pool(name=f"mlpp{l}", bufs=1, space="PSUM") as psp:
                        for j in range(NCH):
                            t0 = j * 512
                            h2 = wkp.tile([128, CT, 512], BF16, tag="h", bufs=2,
                                          name="h2")
                            _ln(nc, psp, wkp, consts, x_tiles, j, h2,
                                lnt["l2w"], lnt["l2b"], apply_lnwb)
                            h1 = [wkp.tile([128, 512], BF16, tag=f"h1_{m}",
                                           name=f"h1_{m}") for m in range(FT)]
                            for qq in range(4):
                                w1q = wkp.tile([128, CT, 512], BF16, tag="w1q",
                                               bufs=2, name="w1q")
                                nc.sync.dma_start(
                                    out=w1q,
                                    in_=w1_d[l].rearrange("(kt p) n -> p kt n",
                                                          p=128)[:, :, qq * 512:(qq + 1) * 512])
                                for mm in range(4):
                                    m = qq * 4 + mm
                                    ph = psp.tile([128, 512], F32, tag="pa",
                                                  bufs=4, name="ph")
                                    for kt in range(CT):
                                        nc.tensor.matmul(
                                            ph, w1q[:, kt, mm * 128:(mm + 1) * 128],
                                            h2[:, kt, :],
                                            start=(kt == 0), stop=(kt == CT - 1))
                                    nc.scalar.activation(
                                        h1[m], ph, AF.Relu,
                                        bias=b1_t[:, m:m + 1])
                            for m2 in range(CT):
                                pw = psp.tile([128, 512], F32, tag="pa", bufs=4,
                                              name="pw")
                                for f in range(FT):
                                    nc.tensor.matmul(
                                        pw, w2_t[:, f, m2 * 128:(m2 + 1) * 128],
                                        h1[f], start=(f == 0),
                                        stop=(not fold_bias and f == FT - 1))
                                if fold_bias:
                                    nc.tensor.matmul(pw, b2_r[0:1, m2 * 128:(m2 + 1) * 128],
                                                     ones_bf[0:1, :], start=False, stop=True)
                                nc.vector.tensor_tensor(
                                    out=x_tiles[m2][:, t0:t0 + 512],
                                    in0=pw,
                                    in1=x_tiles[m2][:, t0:t0 + 512].bitcast(F32),
                                    op=OP.add)

            with tc.tile_pool(name="fin", bufs=1) as wkp, \
                 tc.tile_pool(name="finp", bufs=1, space="PSUM") as psp:
                for j in range(NCH):
                    fin = wkp.tile([128, CT, 512], F32, tag="fin", bufs=2,
                                   name="fin")
                    _ln(nc, psp, wkp, consts, x_tiles, j, fin, lfw_t, lfb_t, apply_lnwb)
                    t0 = j * 512
                    for kt in range(CT):
                        lo, hi = t0, t0 + 512
                        if hi <= T:
                            nc.sync.dma_start(
                                out=out_d[0, kt * 128:(kt + 1) * 128, lo:hi],
                                in_=fin[:, kt, :])
                        elif lo >= T:
                            nc.sync.dma_start(
                                out=out_d[1, kt * 128:(kt + 1) * 128, lo - T:hi - T],
                                in_=fin[:, kt, :])
                        else:
                            cut = T - lo
                            nc.sync.dma_start(
                                out=out_d[0, kt * 128:(kt + 1) * 128, lo:T],
                                in_=fin[:, kt, 0:cut])
                            nc.sync.dma_start(
                                out=out_d[1, kt * 128:(kt + 1) * 128, 0:512 - cut],
                                in_=fin[:, kt, cut:512])
    nc.compile()
    return nc


def _get_nc(fold_bias, apply_lnwb):
    key = (fold_bias, apply_lnwb)
    if key not in _CACHED_NC:
        _CACHED_NC[key] = build_nc(*key)
    return _CACHED_NC[key]


def run(inputs, trace=False):
    zb = all(not np.asarray(inputs[k]).any() for k in
             ("bv", "bo", "b2"))
    idw = (np.all(np.asarray(inputs["ln1_w"]) == 1) and not np.asarray(inputs["ln1_b"]).any()
           and np.all(np.asarray(inputs["ln2_w"]) == 1) and not np.asarray(inputs["ln2_b"]).any()
           and np.all(np.asarray(inputs["lnf_w"]) == 1) and not np.asarray(inputs["lnf_b"]).any())
    nc = _get_nc(not zb, not idw)
    bf = lambda a: np.asarray(a).astype(ml_dtypes.bfloat16)
    f32 = lambda a: np.ascontiguousarray(np.asarray(a), dtype=np.float32)
    img = f32(inputs["image_tensor"]).reshape(16, C, 1024)
    rad = f32(inputs["radar_tensor"]).reshape(16, C, 256)
    posT = np.ascontiguousarray(f32(inputs["pos_emb"])[0].T)
    shared = dict(
        posT=posT,
        Wq=bf(inputs["Wq"]), Wk=bf(inputs["Wk"]), Wv=bf(inputs["Wv"]),
        Wo=bf(inputs["Wo"]), W1=bf(inputs["W1"]), W2=bf(inputs["W2"]),
        bq=f32(inputs["bq"]), bk=f32(inputs["bk"]), bv=bf(inputs["bv"]),
        bo=bf(inputs["bo"]), b1=f32(inputs["b1"]), b2=bf(inputs["b2"]),
        ln1_w=f32(inputs["ln1_w"]), ln1_b=f32(inputs["ln1_b"]),
        ln2_w=f32(inputs["ln2_w"]), ln2_b=f32(inputs["ln2_b"]),
        lnf_w=f32(inputs["lnf_w"]), lnf_b=f32(inputs["lnf_b"]),
    )
    in_maps = []
    for c in range(N_CORES):
        m = dict(shared)
        m["img"] = np.ascontiguousarray(img[2 * c:2 * c + 2])
        m["rad"] = np.ascontiguousarray(rad[2 * c:2 * c + 2])
        in_maps.append(m)
    res = bass_utils.run_bass_kernel_spmd(nc, in_maps,
                                          core_ids=list(range(N_CORES)),
                                          trace=trace)
    xT = np.concatenate([res.results[c]["out_xT"] for c in range(N_CORES)], 0)
    x = np.ascontiguousarray(xT.transpose(0, 2, 1))
    img_out = x[:, :1024, :].reshape(16, C, 16, 64)
    rad_out = x[:, 1024:, :].reshape(16, C, 16, 16)
    return (img_out, rad_out), res


def kernel(**inputs):
    out, _ = run(inputs, trace=False)
    return out
